# revision 31
# baseline (speedup 1.0000x reference)
"""Trainium2 Bass kernel for nn_MultiHeadAttnC (QANet-style self-attention).

Reference computation (per batch b):
    memory = w_mem @ queries[b]          # [2D, L]  (pointwise conv)
    query  = w_query @ queries[b]        # [D, L]
    K, V   = heads of memory             # H=8 heads, DH=16
    Q      = heads of query * DH^-0.5
    S      = Q @ K^T  (masked over kv)   # [H, L, L]
    out[b] = softmax(S) @ V  -> recombined to [D, L]

Strategy (v4):
  - Data parallel: batch b -> NeuronCore b. Weights replicated. No collectives.
  - K-major attention. Heads are split 3/3/2 into three tile groups so a
    S^T tile is [128 kv, ng x 512 q] = ng PSUM banks (ng = 3,3,2): two
    3-bank ring slots double-buffer the exp, and the two leftover banks are
    dedicated AV accumulators. Row-tiled bf16 S matmuls (one bank each)
    stream concurrently (~512 column-cycles per tile).
  - The exp (the roofline: ~27M/core) is SPLIT between the scalar engine
    (native Exp out of PSUM) and the vector engine (single-op Schraudolph:
    i16 = round(x*A + B) == bf16 bits of exp(x); max rel err 3.3%, washes
    out over the ~1600-wide softmax). A greedy time-balancer assigns tiles.
  - AV: ng col-tiled matmuls (M=17: 16 V channels + validity column for the
    softmax denominator) PSUM-accumulated across ALL kv chunks into the
    stream's accumulator bank (start/stop flags): no per-chunk drains, and
    the ring-slot dependency chain is just S -> exp.
  - Masked kv positions are compacted away host-side (exact: zero validity).
  - Per-q-block epilogue: packed reciprocal on 8 denominator rows, broadcast
    DMA, final multiply on GpSimd, partition-permute DMAs for layout.
"""

import numpy as np
from contextlib import ExitStack

import concourse.bass as bass
import concourse.tile as tile
from concourse import bacc, mybir
from concourse import bass_utils

B, D, L, H, DH = 8, 128, 2048, 8, 16
f32 = mybir.dt.float32
bf16 = mybir.dt.bfloat16
i16 = mybir.dt.int16
f32r = mybir.dt.float32r
IN_DT = f32r
QT = 512             # q columns per stream tile
NJQ = L // QT        # 4
NXP = 2              # weight spread planes (4 head-groups each)

# Schraudolph exp constants for round-to-nearest f32->i16 conversion:
# bf16_bits(exp(x)) ~= round(x * 2^7*log2(e) + (127*2^7 - 5.6))
EXP_A = 184.6649652337873
EXP_B = 16250.4

_program_cache: dict = {}


def _body(ctx, tc, qf_d, qkv_d, wq_d, wk_d, wv_d, val_d, out_d, n_kv, compact):
    nc = tc.nc
    Lkv = n_kv * 128
    Exp = mybir.ActivationFunctionType.Exp
    Copy = mybir.ActivationFunctionType.Copy
    mult, add = mybir.AluOpType.mult, mybir.AluOpType.add
    NX = NXP

    consts = ctx.enter_context(tc.tile_pool(name="consts", bufs=1))

    # ---- input DMAs ----
    wq = consts.tile([D, NX, D], IN_DT, tag="wq")
    wk = consts.tile([D, NX, D], IN_DT, tag="wk")
    for X in range(NX):
        nc.sync.dma_start(out=wk[:, X, :], in_=wk_d[X])
        nc.sync.dma_start(out=wq[:, X, :], in_=wq_d[X])
    wv = consts.tile([D, D], IN_DT, tag="wv")
    nc.sync.dma_start(out=wv, in_=wv_d)
    qkv = consts.tile([D, Lkv], IN_DT, tag="qkv")
    qf = consts.tile([D, L], IN_DT, tag="qf")
    qs_list = [(qkv, qkv_d, c, min(512, Lkv - c))
               for c in range(0, Lkv, 512)]
    qf_list = [(qf, qf_d, j * QT, QT) for j in range(NJQ)]
    # interleave kv/q blocks and alternate queues so both proj streams can
    # start early and neither DMA ring serializes the other
    order = []
    for a, b in zip(qs_list, qf_list + [None] * 9):
        order.append(a)
        if b:
            order.append(b)
    for i, (t, d, c, n) in enumerate(order):
        eng = nc.gpsimd if i % 2 == 0 else nc.sync
        eng.dma_start(out=t[:, c:c + n], in_=d[:, c:c + n])

    q_sp = consts.tile([D, NX, L], bf16, tag="q_sp")
    k_sp = consts.tile([D, NX, Lkv], bf16, tag="k_sp")
    v_sb = consts.tile([128, n_kv, H, DH + 1], bf16, tag="v_sb")
    out_sb = consts.tile([D, L], f32, tag="out_sb")

    if compact:
        # validity (pre-broadcast per head host-side) -> ones column of v_sb
        # via DVE strided write (DMA would clobber neighbors: 2-byte elems)
        val16 = consts.tile([128, n_kv * H], bf16, tag="val16")
        nc.sync.dma_start(out=val16, in_=val_d)
        dst = bass.AP(tensor=v_sb.tensor, offset=v_sb.offset + DH,
                      ap=[[n_kv * H * (DH + 1), 128], [DH + 1, n_kv * H]])
        nc.vector.tensor_copy(out=dst, in_=val16)
    else:
        val = consts.tile([128, n_kv], f32, tag="val")
        nc.gpsimd.dma_start(out=val, in_=val_d)
        ones8 = consts.tile([128, 8], f32, tag="ones8")
        nc.vector.memset(ones8, 1.0)

    # ---- PSUM: 3 duo ring slots (2 banks) + 2 AV accumulator banks ----
    ring = ctx.enter_context(tc.tile_pool(name="ring", bufs=3, space="PSUM"))
    accp = ctx.enter_context(tc.tile_pool(name="accp", bufs=2, space="PSUM"))

    def s_tile():
        return ring.tile([128, 2 * QT], f32, tag="s", name="s")

    # ---- HAM warmup + ACT exp-table prime ----
    warm_in = consts.tile([128, 512], bf16, tag="warm_in")
    nc.vector.memset(warm_in, 0.0)
    wps = accp.tile([128, 512], f32, tag="acc", name="acc")
    for i in range(3):
        nc.tensor.matmul(wps[:, 0:512], lhsT=warm_in[:, 0:128],
                         rhs=warm_in, start=True, stop=True)
    p_warm = consts.tile([128, 128], bf16, tag="p_warm")
    nc.scalar.activation(out=p_warm, in_=warm_in[:, 0:128], func=Exp)

    # ---- projections: PSUM->SBUF evacuation alternates scalar/vector ----
    copy_flip = [0]

    def evac(dst_ap, src_ap):
        copy_flip[0] ^= 1
        if copy_flip[0]:
            nc.scalar.activation(out=dst_ap, in_=src_ap, func=Copy)
        else:
            nc.vector.tensor_copy(out=dst_ap, in_=src_ap)

    def proj_k(X):
        col = 0
        while col < Lkv:
            n = min(2 * QT, Lkv - col)
            ps = s_tile()
            for off in range(0, n, 512):
                m = min(512, n - off)
                nc.tensor.matmul(ps[:, off:off + m], lhsT=wk[:, X, :],
                                 rhs=qkv[:, col + off:col + off + m],
                                 start=True, stop=True)
            evac(k_sp[:, X, col:col + n], ps[:, 0:n])
            col += n

    def proj_q(X):
        col = 0
        while col < L:
            n = min(2 * QT, L - col)
            ps = s_tile()
            for off in range(0, n, 512):
                m = min(512, n - off)
                nc.tensor.matmul(ps[:, off:off + m], lhsT=wq[:, X, :],
                                 rhs=qf[:, col + off:col + off + m],
                                 start=True, stop=True)
            evac(q_sp[:, X, col:col + n], ps[:, 0:n])
            col += n

    def proj_v():
        for c in range(n_kv):
            vp = accp.tile([128, 512], f32, tag="acc", name="acc")
            nc.tensor.matmul(vp[:, 0:D], lhsT=qkv[:, c * 128:(c + 1) * 128],
                             rhs=wv, start=True, stop=True)
            if compact:
                nc.vector.tensor_copy(
                    out=v_sb[:, c, :, 0:DH],
                    in_=vp[:, 0:D].rearrange("p (h x) -> p h x", x=DH))
            else:
                nc.vector.tensor_scalar_mul(
                    v_sb[:, c, :, 0:DH],
                    vp[:, 0:D].rearrange("p (h x) -> p h x", x=DH),
                    val[:, c:c + 1])
                nc.vector.tensor_scalar_mul(
                    v_sb[:, c, :, DH:DH + 1],
                    ones8.rearrange("p (h x) -> p h x", x=1),
                    val[:, c:c + 1])

    proj_k(0)
    proj_q(0)
    proj_v()

    # ---- attention ----
    p_act = ctx.enter_context(tc.tile_pool(name="p_act", bufs=9))
    p_dve = ctx.enter_context(tc.tile_pool(name="p_dve", bufs=9))
    a_pool = ctx.enter_context(tc.tile_pool(name="a_pool", bufs=4))
    misc = ctx.enter_context(tc.tile_pool(name="misc", bufs=2))

    drains = {}
    # greedy exp-engine balancer (ns accumulators; DVE starts with its misc)
    eng_t = {"act": 0.0, "dve": 12000.0}

    def stream_pair(jq, X):
        """Both head-pair streams (j=0: groups 0,1 / j=1: groups 2,3) of one
        (jq, X) advance chunk-by-chunk together: their S duos use disjoint
        row-groups and their AV duos disjoint col-groups, so the PE streams
        4 matmuls concurrently (quad efficiency) while each exp tile stays
        2 banks (duo) for the 3-slot ring + 2 accumulator-bank layout."""
        n = 2 * QT
        qs = slice(jq * QT, (jq + 1) * QT)
        accs = {j: accp.tile([128, QT], f32, tag="acc", name="acc")
                for j in (0, 1)}

        def s_duo(c, j):
            ck = slice(c * 128, (c + 1) * 128)
            sp = s_tile()
            for gi in range(2):
                g = 2 * j + gi
                nc.tensor.matmul(
                    sp[:, gi * QT:(gi + 1) * QT],
                    lhsT=k_sp[g * 32:(g + 1) * 32, X, ck],
                    rhs=q_sp[g * 32:(g + 1) * 32, X, qs],
                    start=True, stop=True, tile_position=(g * 32, 0))
            return sp

        sps, ps, avq = {}, {}, []

        def emit_av(c):
            st, en = (c == 0), (c == n_kv - 1)
            for j in (0, 1):
                rhs_p = ps.pop((c, j))
                for gi in range(2):
                    g = 2 * j + gi
                    nc.tensor.matmul(
                        accs[j][g * 32:g * 32 + DH + 1, :],
                        lhsT=v_sb[:, c, 4 * X + g, :],
                        rhs=rhs_p[:, gi * QT:(gi + 1) * QT],
                        start=st, stop=en, tile_position=(0, g * 32))

        for j in (0, 1):
            sps[(0, j)] = s_duo(0, j)
        for c in range(n_kv):
            for j in (0, 1):
                sp = sps.pop((c, j))
                c_act = eng_t["act"] + n / 1.2 + 290
                c_dve = eng_t["dve"] + (n / 0.96 + 160) * 1.45
                if c_act <= c_dve:
                    eng_t["act"] = c_act
                    p = p_act.tile([128, 2 * QT], bf16, tag="p")
                    nc.scalar.activation(out=p, in_=sp, func=Exp)
                    ps[(c, j)] = p
                else:
                    eng_t["dve"] = c_dve
                    p16 = p_dve.tile([128, 2 * QT], i16, tag="p16")
                    nc.vector.tensor_scalar(out=p16, in0=sp,
                                            scalar1=EXP_A, scalar2=EXP_B,
                                            op0=mult, op1=add)
                    ps[(c, j)] = p16.bitcast(bf16)
            if c + 1 < n_kv:
                for j in (0, 1):
                    sps[(c + 1, j)] = s_duo(c + 1, j)
            avq.append(c)
            if len(avq) > 5:
                emit_av(avq.pop(0))
        while avq:
            emit_av(avq.pop(0))
        for j in (0, 1):
            a_sb = a_pool.tile([64, QT], f32, tag=f"a{X}{j}", name="a")
            base = 64 * j
            nc.vector.tensor_copy(out=a_sb, in_=accs[j][base:base + 64, :])
            eng_t["dve"] += QT / 0.96 + 160
            drains[(jq, X, j)] = a_sb

    dmaq = [nc.gpsimd, nc.sync]

    def epilogue(jq):
        qs = slice(jq * QT, (jq + 1) * QT)
        pk = misc.tile([8, QT], f32, tag="pk")
        qi = [0]

        def gdma(out, in_):
            qi[0] = (qi[0] + 1) % len(dmaq)
            dmaq[qi[0]].dma_start(out=out, in_=in_)

        for X in range(2):
            for j in range(2):
                a_sb = drains[(jq, X, j)]
                for gi in range(2):
                    h = 4 * X + 2 * j + gi
                    gdma(pk[h:h + 1, :],
                         a_sb[gi * 32 + DH:gi * 32 + DH + 1, :])
        rec = misc.tile([8, QT], f32, tag="rec")
        nc.vector.reciprocal_approx_fast(out=rec, in_=pk)
        eng_t["dve"] += 2 * QT / 0.96 + 160
        rb = misc.tile([128, QT], f32, tag="rb")
        nc.sync.dma_start(
            out=rb,
            in_=bass.AP(tensor=rec.tensor, offset=rec.offset,
                        ap=[[QT, 8], [0, DH], [1, QT]]))
        xt = misc.tile([128, QT], f32, tag="xt")
        for X in range(2):
            for j in range(2):
                a_sb = drains[(jq, X, j)]
                for gi in range(2):
                    h = 4 * X + 2 * j + gi
                    gdma(xt[h * DH:(h + 1) * DH, :],
                         a_sb[gi * 32:gi * 32 + DH, :])
        nc.gpsimd.tensor_mul(out=out_sb[:, qs], in0=xt, in1=rb)
        nc.sync.dma_start(out=out_d[:, qs], in_=out_sb[:, qs])

    for jq in range(NJQ):
        stream_pair(jq, 0)
        if jq == 0:
            proj_k(1)
            proj_q(1)
    for jq in range(NJQ):
        stream_pair(jq, 1)
        epilogue(jq)


def _build(n_kv: int, compact: bool) -> "bacc.Bacc":
    Lkv = n_kv * 128
    NX = NXP
    nc = bacc.Bacc("TRN2", target_bir_lowering=False, debug=False,
                   enable_asserts=True, num_devices=B)
    qf_d = nc.dram_tensor("q_full", [D, L], IN_DT, kind="ExternalInput").ap()
    qkv_d = nc.dram_tensor("q_kv", [D, Lkv], IN_DT, kind="ExternalInput").ap()
    wq_d = nc.dram_tensor("wq_sp", [NX, D, D], IN_DT, kind="ExternalInput").ap()
    wk_d = nc.dram_tensor("wk_sp", [NX, D, D], IN_DT, kind="ExternalInput").ap()
    wv_d = nc.dram_tensor("wv_t", [D, D], IN_DT, kind="ExternalInput").ap()
    val_dt = bf16 if compact else f32
    val_shape = [128, n_kv * H] if compact else [128, n_kv]
    val_d = nc.dram_tensor("valid", val_shape, val_dt,
                           kind="ExternalInput").ap()
    out_d = nc.dram_tensor("out", [D, L], f32, kind="ExternalOutput").ap()

    with tile.TileContext(nc) as tc, ExitStack() as ctx:
        _body(ctx, tc, qf_d, qkv_d, wq_d, wk_d, wv_d, val_d, out_d, n_kv,
              compact)
    nc.compile()
    return nc


def _prep_weights(w_mem: np.ndarray, w_query: np.ndarray):
    """Spread head weights into 32-row tile groups (rows 16:32 zero) across
    two planes of 4 head-groups, pre-transposed for use as matmul lhsT.
    Q gets the DH^-0.5 scale."""
    wq_sp = np.zeros((NXP, D, D), np.float32)
    wk_sp = np.zeros((NXP, D, D), np.float32)
    scale = np.float32(DH ** -0.5)
    for X in range(NXP):
        for g in range(4):
            h = 4 * X + g
            wq_sp[X][:, 32 * g:32 * g + DH] = (w_query[DH * h:DH * (h + 1), :] * scale).T
            wk_sp[X][:, 32 * g:32 * g + DH] = w_mem[DH * h:DH * (h + 1), :].T
    wv_t = np.ascontiguousarray(w_mem[D:2 * D, :].T)
    return wq_sp, wk_sp, wv_t


COMPACT_KV = True  # drop masked kv positions host-side (exact: they get a
                   # zero validity column -> contribute 0 to num and denom)


def prepare(queries: np.ndarray, mask: np.ndarray, w_mem: np.ndarray,
            w_query: np.ndarray):
    """Build (compiled program, per-core input maps)."""
    import ml_dtypes
    assert queries.shape == (B, D, L) and mask.shape == (B, L)
    maskf = mask.astype(np.float32)
    kept = [np.nonzero(maskf[b] > 0.0)[0] for b in range(B)]
    if COMPACT_KV and all(len(k) > 0 for k in kept):
        n_kv = max(1, -(-max(len(k) for k in kept) // 128))
        compact = True
    else:
        n_kv = L // 128
        kept = None
        compact = False
    Lkv = n_kv * 128

    key = (n_kv, compact)
    nc = _program_cache.get(key)
    if nc is None:
        nc = _program_cache[key] = _build(n_kv, compact)

    wq_sp, wk_sp, wv_t = _prep_weights(
        w_mem.astype(np.float32), w_query.astype(np.float32))

    in_maps = []
    for b in range(B):
        qb = np.ascontiguousarray(queries[b], dtype=np.float32)
        if kept is not None:
            idx = kept[b]
            qkv = np.zeros((D, Lkv), np.float32)
            qkv[:, :len(idx)] = qb[:, idx]
            val = np.zeros(Lkv, np.float32)
            val[:len(idx)] = 1.0
        else:
            qkv = qb
            val = maskf[b]
        valT = np.ascontiguousarray(val.reshape(n_kv, 128).T)
        in_maps.append({
            "q_full": qb,
            "q_kv": np.ascontiguousarray(qkv),
            "wq_sp": wq_sp,
            "wk_sp": wk_sp,
            "wv_t": wv_t,
            "valid": (np.ascontiguousarray(np.repeat(valT, H, axis=1))
                      .astype(ml_dtypes.bfloat16) if compact else valT),
        })
    return nc, in_maps


def kernel(queries: np.ndarray, mask: np.ndarray, w_mem: np.ndarray,
           w_query: np.ndarray) -> np.ndarray:
    nc, in_maps = prepare(queries, mask, w_mem, w_query)
    res = bass_utils.run_bass_kernel_spmd(nc, in_maps, core_ids=list(range(B)))
    return np.stack([res.results[b]["out"] for b in range(B)]).astype(np.float32)


# revision 32
# speedup vs baseline: 1.0448x; 1.0448x over previous
"""Trainium2 Bass kernel for nn_MultiHeadAttnC (QANet-style self-attention).

Reference computation (per batch b):
    memory = w_mem @ queries[b]          # [2D, L]  (pointwise conv)
    query  = w_query @ queries[b]        # [D, L]
    K, V   = heads of memory             # H=8 heads, DH=16
    Q      = heads of query * DH^-0.5
    S      = Q @ K^T  (masked over kv)   # [H, L, L]
    out[b] = softmax(S) @ V  -> recombined to [D, L]

Strategy (v4):
  - Data parallel: batch b -> NeuronCore b. Weights replicated. No collectives.
  - K-major attention. Heads are split 3/3/2 into three tile groups so a
    S^T tile is [128 kv, ng x 512 q] = ng PSUM banks (ng = 3,3,2): two
    3-bank ring slots double-buffer the exp, and the two leftover banks are
    dedicated AV accumulators. Row-tiled bf16 S matmuls (one bank each)
    stream concurrently (~512 column-cycles per tile).
  - The exp (the roofline: ~27M/core) is SPLIT between the scalar engine
    (native Exp out of PSUM) and the vector engine (single-op Schraudolph:
    i16 = round(x*A + B) == bf16 bits of exp(x); max rel err 3.3%, washes
    out over the ~1600-wide softmax). A greedy time-balancer assigns tiles.
  - AV: ng col-tiled matmuls (M=17: 16 V channels + validity column for the
    softmax denominator) PSUM-accumulated across ALL kv chunks into the
    stream's accumulator bank (start/stop flags): no per-chunk drains, and
    the ring-slot dependency chain is just S -> exp.
  - Masked kv positions are compacted away host-side (exact: zero validity).
  - Per-q-block epilogue: packed reciprocal on 8 denominator rows, broadcast
    DMA, final multiply on GpSimd, partition-permute DMAs for layout.
"""

import numpy as np
from contextlib import ExitStack

import concourse.bass as bass
import concourse.tile as tile
from concourse import bacc, mybir
from concourse import bass_utils

B, D, L, H, DH = 8, 128, 2048, 8, 16
f32 = mybir.dt.float32
bf16 = mybir.dt.bfloat16
i16 = mybir.dt.int16
f32r = mybir.dt.float32r
IN_DT = f32r
QT = 512             # q columns per stream tile
NJQ = L // QT        # 4
NXP = 2              # weight spread planes (4 head-groups each)

# Schraudolph exp constants for round-to-nearest f32->i16 conversion:
# bf16_bits(exp(x)) ~= round(x * 2^7*log2(e) + (127*2^7 - 5.6))
EXP_A = 184.6649652337873
EXP_B = 16250.4

_program_cache: dict = {}


def _body(ctx, tc, qf_d, qkv_d, wq_d, wk_d, wv_d, val_d, out_d, n_kv, compact):
    nc = tc.nc
    Lkv = n_kv * 128
    Exp = mybir.ActivationFunctionType.Exp
    Copy = mybir.ActivationFunctionType.Copy
    mult, add = mybir.AluOpType.mult, mybir.AluOpType.add
    NX = NXP

    consts = ctx.enter_context(tc.tile_pool(name="consts", bufs=1))

    # ---- input DMAs ----
    wq = consts.tile([D, NX, D], IN_DT, tag="wq")
    wk = consts.tile([D, NX, D], IN_DT, tag="wk")
    for X in range(NX):
        nc.sync.dma_start(out=wk[:, X, :], in_=wk_d[X])
        nc.sync.dma_start(out=wq[:, X, :], in_=wq_d[X])
    wv = consts.tile([D, D], IN_DT, tag="wv")
    nc.sync.dma_start(out=wv, in_=wv_d)
    qkv = consts.tile([D, Lkv], IN_DT, tag="qkv")
    qf = consts.tile([D, L], IN_DT, tag="qf")
    qs_list = [(qkv, qkv_d, c, min(512, Lkv - c))
               for c in range(0, Lkv, 512)]
    qf_list = [(qf, qf_d, j * QT, QT) for j in range(NJQ)]
    # interleave kv/q blocks and alternate queues so both proj streams can
    # start early and neither DMA ring serializes the other
    order = []
    for a, b in zip(qs_list, qf_list + [None] * 9):
        order.append(a)
        if b:
            order.append(b)
    for i, (t, d, c, n) in enumerate(order):
        eng = nc.gpsimd if i % 2 == 0 else nc.sync
        eng.dma_start(out=t[:, c:c + n], in_=d[:, c:c + n])

    q_sp = consts.tile([D, NX, L], bf16, tag="q_sp")
    k_sp = consts.tile([D, NX, Lkv], bf16, tag="k_sp")
    v_sb = consts.tile([128, n_kv, H, DH + 1], bf16, tag="v_sb")
    out_sb = consts.tile([D, L], f32, tag="out_sb")

    if compact:
        # validity (pre-broadcast per head host-side) -> ones column of v_sb
        # via DVE strided write (DMA would clobber neighbors: 2-byte elems)
        val16 = consts.tile([128, n_kv * H], bf16, tag="val16")
        nc.sync.dma_start(out=val16, in_=val_d)
        dst = bass.AP(tensor=v_sb.tensor, offset=v_sb.offset + DH,
                      ap=[[n_kv * H * (DH + 1), 128], [DH + 1, n_kv * H]])
        nc.vector.tensor_copy(out=dst, in_=val16)
    else:
        val = consts.tile([128, n_kv], f32, tag="val")
        nc.gpsimd.dma_start(out=val, in_=val_d)
        ones8 = consts.tile([128, 8], f32, tag="ones8")
        nc.vector.memset(ones8, 1.0)

    # ---- PSUM: 3 duo ring slots (2 banks) + 2 AV accumulator banks ----
    ring = ctx.enter_context(tc.tile_pool(name="ring", bufs=3, space="PSUM"))
    accp = ctx.enter_context(tc.tile_pool(name="accp", bufs=2, space="PSUM"))

    def s_tile():
        return ring.tile([128, 2 * QT], f32, tag="s", name="s")

    # ---- HAM warmup + ACT exp-table prime ----
    warm_in = consts.tile([128, 512], bf16, tag="warm_in")
    nc.vector.memset(warm_in, 0.0)
    wps = accp.tile([128, 512], f32, tag="acc", name="acc")
    for i in range(3):
        nc.tensor.matmul(wps[:, 0:512], lhsT=warm_in[:, 0:128],
                         rhs=warm_in, start=True, stop=True)
    p_warm = consts.tile([128, 128], bf16, tag="p_warm")
    nc.scalar.activation(out=p_warm, in_=warm_in[:, 0:128], func=Exp)

    # ---- projections: PSUM->SBUF evacuation alternates scalar/vector ----
    copy_flip = [0]

    def evac(dst_ap, src_ap):
        copy_flip[0] ^= 1
        if copy_flip[0]:
            nc.scalar.activation(out=dst_ap, in_=src_ap, func=Copy)
        else:
            nc.vector.tensor_copy(out=dst_ap, in_=src_ap)

    def proj_k(X):
        col = 0
        while col < Lkv:
            n = min(2 * QT, Lkv - col)
            ps = s_tile()
            for off in range(0, n, 512):
                m = min(512, n - off)
                nc.tensor.matmul(ps[:, off:off + m], lhsT=wk[:, X, :],
                                 rhs=qkv[:, col + off:col + off + m],
                                 start=True, stop=True)
            evac(k_sp[:, X, col:col + n], ps[:, 0:n])
            col += n

    def proj_q(X):
        col = 0
        while col < L:
            n = min(2 * QT, L - col)
            ps = s_tile()
            for off in range(0, n, 512):
                m = min(512, n - off)
                nc.tensor.matmul(ps[:, off:off + m], lhsT=wq[:, X, :],
                                 rhs=qf[:, col + off:col + off + m],
                                 start=True, stop=True)
            evac(q_sp[:, X, col:col + n], ps[:, 0:n])
            col += n

    def proj_v():
        for c in range(n_kv):
            vp = accp.tile([128, 512], f32, tag="acc", name="acc")
            nc.tensor.matmul(vp[:, 0:D], lhsT=qkv[:, c * 128:(c + 1) * 128],
                             rhs=wv, start=True, stop=True)
            if compact:
                nc.vector.tensor_copy(
                    out=v_sb[:, c, :, 0:DH],
                    in_=vp[:, 0:D].rearrange("p (h x) -> p h x", x=DH))
            else:
                nc.vector.tensor_scalar_mul(
                    v_sb[:, c, :, 0:DH],
                    vp[:, 0:D].rearrange("p (h x) -> p h x", x=DH),
                    val[:, c:c + 1])
                nc.vector.tensor_scalar_mul(
                    v_sb[:, c, :, DH:DH + 1],
                    ones8.rearrange("p (h x) -> p h x", x=1),
                    val[:, c:c + 1])

    proj_k(0)
    proj_q(0)
    proj_v()

    # ---- attention ----
    p_act = ctx.enter_context(tc.tile_pool(name="p_act", bufs=6))
    p_dve = ctx.enter_context(tc.tile_pool(name="p_dve", bufs=6))
    a_pool = ctx.enter_context(tc.tile_pool(name="a_pool", bufs=4))
    misc = ctx.enter_context(tc.tile_pool(name="misc", bufs=2))

    drains = {}
    # greedy exp-engine balancer (ns accumulators; DVE starts with its misc)
    eng_t = {"act": 0.0, "dve": 12000.0}

    def stream_pair(jq, X):
        """Both head-pair streams (j=0: groups 0,1 / j=1: groups 2,3) of one
        (jq, X) advance chunk-by-chunk together: their S duos use disjoint
        row-groups and their AV duos disjoint col-groups, so the PE streams
        4 matmuls concurrently (quad efficiency) while each exp tile stays
        2 banks (duo) for the 3-slot ring + 2 accumulator-bank layout."""
        n = 2 * QT
        qs = slice(jq * QT, (jq + 1) * QT)
        accs = {j: accp.tile([128, QT], f32, tag="acc", name="acc")
                for j in (0, 1)}

        def s_duo(c, j):
            ck = slice(c * 128, (c + 1) * 128)
            sp = s_tile()
            for gi in range(2):
                g = 2 * j + gi
                nc.tensor.matmul(
                    sp[:, gi * QT:(gi + 1) * QT],
                    lhsT=k_sp[g * 32:(g + 1) * 32, X, ck],
                    rhs=q_sp[g * 32:(g + 1) * 32, X, qs],
                    start=True, stop=True, tile_position=(g * 32, 0))
            return sp

        sps, ps, avq = {}, {}, []

        def emit_av(c):
            st, en = (c == 0), (c == n_kv - 1)
            for j in (0, 1):
                rhs_p = ps.pop((c, j))
                for gi in range(2):
                    g = 2 * j + gi
                    nc.tensor.matmul(
                        accs[j][g * 32:g * 32 + DH + 1, :],
                        lhsT=v_sb[:, c, 4 * X + g, :],
                        rhs=rhs_p[:, gi * QT:(gi + 1) * QT],
                        start=st, stop=en, tile_position=(0, g * 32))

        for j in (0, 1):
            sps[(0, j)] = s_duo(0, j)
        for c in range(n_kv):
            for j in (0, 1):
                sp = sps.pop((c, j))
                c_act = eng_t["act"] + n / 1.2 + 290
                c_dve = eng_t["dve"] + (n / 0.96 + 160) * 1.2
                if c_act <= c_dve:
                    eng_t["act"] = c_act
                    p = p_act.tile([128, 2 * QT], bf16, tag="p")
                    nc.scalar.activation(out=p, in_=sp, func=Exp)
                    ps[(c, j)] = p
                else:
                    eng_t["dve"] = c_dve
                    p16 = p_dve.tile([128, 2 * QT], i16, tag="p16")
                    nc.vector.tensor_scalar(out=p16, in0=sp,
                                            scalar1=EXP_A, scalar2=EXP_B,
                                            op0=mult, op1=add)
                    ps[(c, j)] = p16.bitcast(bf16)
            if c + 1 < n_kv:
                for j in (0, 1):
                    sps[(c + 1, j)] = s_duo(c + 1, j)
            avq.append(c)
            if len(avq) > 3:
                emit_av(avq.pop(0))
        while avq:
            emit_av(avq.pop(0))
        for j in (0, 1):
            a_sb = a_pool.tile([64, QT], f32, tag=f"a{X}{j}", name="a")
            base = 64 * j
            nc.vector.tensor_copy(out=a_sb, in_=accs[j][base:base + 64, :])
            eng_t["dve"] += QT / 0.96 + 160
            drains[(jq, X, j)] = a_sb

    dmaq = [nc.gpsimd, nc.sync]

    def epilogue(jq):
        qs = slice(jq * QT, (jq + 1) * QT)
        pk = misc.tile([8, QT], f32, tag="pk")
        qi = [0]

        def gdma(out, in_):
            qi[0] = (qi[0] + 1) % len(dmaq)
            dmaq[qi[0]].dma_start(out=out, in_=in_)

        for X in range(2):
            for j in range(2):
                a_sb = drains[(jq, X, j)]
                for gi in range(2):
                    h = 4 * X + 2 * j + gi
                    gdma(pk[h:h + 1, :],
                         a_sb[gi * 32 + DH:gi * 32 + DH + 1, :])
        rec = misc.tile([8, QT], f32, tag="rec")
        nc.vector.reciprocal_approx_fast(out=rec, in_=pk)
        eng_t["dve"] += 2 * QT / 0.96 + 160
        rb = misc.tile([128, QT], f32, tag="rb")
        nc.sync.dma_start(
            out=rb,
            in_=bass.AP(tensor=rec.tensor, offset=rec.offset,
                        ap=[[QT, 8], [0, DH], [1, QT]]))
        xt = misc.tile([128, QT], f32, tag="xt")
        for X in range(2):
            for j in range(2):
                a_sb = drains[(jq, X, j)]
                for gi in range(2):
                    h = 4 * X + 2 * j + gi
                    gdma(xt[h * DH:(h + 1) * DH, :],
                         a_sb[gi * 32:gi * 32 + DH, :])
        nc.gpsimd.tensor_mul(out=out_sb[:, qs], in0=xt, in1=rb)
        nc.sync.dma_start(out=out_d[:, qs], in_=out_sb[:, qs])

    for jq in range(NJQ):
        stream_pair(jq, 0)
        if jq == 0:
            proj_k(1)
            proj_q(1)
    for jq in range(NJQ):
        stream_pair(jq, 1)
        epilogue(jq)


def _build(n_kv: int, compact: bool) -> "bacc.Bacc":
    Lkv = n_kv * 128
    NX = NXP
    nc = bacc.Bacc("TRN2", target_bir_lowering=False, debug=False,
                   enable_asserts=True, num_devices=B)
    qf_d = nc.dram_tensor("q_full", [D, L], IN_DT, kind="ExternalInput").ap()
    qkv_d = nc.dram_tensor("q_kv", [D, Lkv], IN_DT, kind="ExternalInput").ap()
    wq_d = nc.dram_tensor("wq_sp", [NX, D, D], IN_DT, kind="ExternalInput").ap()
    wk_d = nc.dram_tensor("wk_sp", [NX, D, D], IN_DT, kind="ExternalInput").ap()
    wv_d = nc.dram_tensor("wv_t", [D, D], IN_DT, kind="ExternalInput").ap()
    val_dt = bf16 if compact else f32
    val_shape = [128, n_kv * H] if compact else [128, n_kv]
    val_d = nc.dram_tensor("valid", val_shape, val_dt,
                           kind="ExternalInput").ap()
    out_d = nc.dram_tensor("out", [D, L], f32, kind="ExternalOutput").ap()

    with tile.TileContext(nc) as tc, ExitStack() as ctx:
        _body(ctx, tc, qf_d, qkv_d, wq_d, wk_d, wv_d, val_d, out_d, n_kv,
              compact)
    nc.compile()
    return nc


def _prep_weights(w_mem: np.ndarray, w_query: np.ndarray):
    """Spread head weights into 32-row tile groups (rows 16:32 zero) across
    two planes of 4 head-groups, pre-transposed for use as matmul lhsT.
    Q gets the DH^-0.5 scale."""
    wq_sp = np.zeros((NXP, D, D), np.float32)
    wk_sp = np.zeros((NXP, D, D), np.float32)
    scale = np.float32(DH ** -0.5)
    for X in range(NXP):
        for g in range(4):
            h = 4 * X + g
            wq_sp[X][:, 32 * g:32 * g + DH] = (w_query[DH * h:DH * (h + 1), :] * scale).T
            wk_sp[X][:, 32 * g:32 * g + DH] = w_mem[DH * h:DH * (h + 1), :].T
    wv_t = np.ascontiguousarray(w_mem[D:2 * D, :].T)
    return wq_sp, wk_sp, wv_t


COMPACT_KV = True  # drop masked kv positions host-side (exact: they get a
                   # zero validity column -> contribute 0 to num and denom)


def prepare(queries: np.ndarray, mask: np.ndarray, w_mem: np.ndarray,
            w_query: np.ndarray):
    """Build (compiled program, per-core input maps)."""
    import ml_dtypes
    assert queries.shape == (B, D, L) and mask.shape == (B, L)
    maskf = mask.astype(np.float32)
    kept = [np.nonzero(maskf[b] > 0.0)[0] for b in range(B)]
    if COMPACT_KV and all(len(k) > 0 for k in kept):
        n_kv = max(1, -(-max(len(k) for k in kept) // 128))
        compact = True
    else:
        n_kv = L // 128
        kept = None
        compact = False
    Lkv = n_kv * 128

    key = (n_kv, compact)
    nc = _program_cache.get(key)
    if nc is None:
        nc = _program_cache[key] = _build(n_kv, compact)

    wq_sp, wk_sp, wv_t = _prep_weights(
        w_mem.astype(np.float32), w_query.astype(np.float32))

    in_maps = []
    for b in range(B):
        qb = np.ascontiguousarray(queries[b], dtype=np.float32)
        if kept is not None:
            idx = kept[b]
            qkv = np.zeros((D, Lkv), np.float32)
            qkv[:, :len(idx)] = qb[:, idx]
            val = np.zeros(Lkv, np.float32)
            val[:len(idx)] = 1.0
        else:
            qkv = qb
            val = maskf[b]
        valT = np.ascontiguousarray(val.reshape(n_kv, 128).T)
        in_maps.append({
            "q_full": qb,
            "q_kv": np.ascontiguousarray(qkv),
            "wq_sp": wq_sp,
            "wk_sp": wk_sp,
            "wv_t": wv_t,
            "valid": (np.ascontiguousarray(np.repeat(valT, H, axis=1))
                      .astype(ml_dtypes.bfloat16) if compact else valT),
        })
    return nc, in_maps


def kernel(queries: np.ndarray, mask: np.ndarray, w_mem: np.ndarray,
           w_query: np.ndarray) -> np.ndarray:
    nc, in_maps = prepare(queries, mask, w_mem, w_query)
    res = bass_utils.run_bass_kernel_spmd(nc, in_maps, core_ids=list(range(B)))
    return np.stack([res.results[b]["out"] for b in range(B)]).astype(np.float32)
